# revision 11
# baseline (speedup 1.0000x reference)
"""Trainium2 Bass kernel for nn_BilinearBlock (bilinear attention + bilinear MLP block).

Sharding: 8 cores = (batch b in 0..3) x (zigzag half h in 0..1).
Queries are sharded causally-balanced: in 256-row blocks g of the sequence,
core h=0 takes g in [0,2,5,7], h=1 takes [1,3,4,6].  Program position
p=0..3 statically processes the first KBC[p]=[4,8,12,16] key blocks of 128;
each core's assigned g satisfies 2g+2 <= KBC[p] (no missing keys) and
2g >= 4p (blocks below the masked window are fully allowed on all cores),
so one SPMD program serves both cores while skipping ~38% of masked score
work.  The causal mask is applied (as a multiply) only to the 4 diagonal
key blocks of each position.

Everything on-device is kept feature-major ("T layout": features/head dims
on SBUF partitions, sequence positions on the free axis).  RMSNorm factors
are computed via ones-matmul partition reductions and applied to the rope
tables (q/k) and v, broadcast across partitions with gpsimd.

The MLP runs in fp8e4 DoubleRow (2x PE throughput): wm/wn/wp arrive
pre-scaled by 8 in fp8; xn2 = out1 * r2 is quantized on the fly; the x64
scale of the wp product is undone in the final PSUM->SBUF copy.
"""
import os
import sys

for _p in ('/opt/trn_rl_repo',):
    if _p not in sys.path:
        sys.path.insert(0, _p)

import numpy as np
import ml_dtypes

import concourse.bass as bass
import concourse.mybir as mybir
import concourse.tile as tile
from concourse import bacc
from concourse.bass_utils import run_bass_kernel_spmd
from concourse.masks import make_identity

P = 128
S = 2048          # full sequence
R = 1024          # query rows per core
D = 1024          # d_model
DH = 128          # d_head
DM = 4096         # d_mlp
NT = 512          # projection block width
QT = 256          # query tile width (one position)
FC = D // P       # 8 feature chunks
EPS = 1e-6
F32 = mybir.dt.float32
F32R = mybir.dt.float32r
BF16 = mybir.dt.bfloat16
FP8 = mybir.dt.float8e4

GSET = [[0, 2, 5, 7], [1, 3, 4, 6]]   # 256-row q blocks per zigzag half
KBC = [4, 8, 12, 16]                  # static key-block count per position

LAST_EXEC_NS = None

_cached = {}


def _build():
    nc = bacc.Bacc("TRN2", target_bir_lowering=False, debug=False, num_devices=8)

    xT = nc.dram_tensor("xT", [D, S], F32R, kind="ExternalInput").ap()
    xqT = nc.dram_tensor("xqT", [D, R], F32R, kind="ExternalInput").ap()
    cos_kv = nc.dram_tensor("cos_kv", [DH, S], F32, kind="ExternalInput").ap()
    sin_kv = nc.dram_tensor("sin_kv", [DH, S], F32, kind="ExternalInput").ap()
    cos_q = nc.dram_tensor("cos_q", [DH, R], F32, kind="ExternalInput").ap()
    sin_q = nc.dram_tensor("sin_q", [DH, R], F32, kind="ExternalInput").ap()
    maskP = nc.dram_tensor("maskP", [16 * P, QT], BF16, kind="ExternalInput").ap()
    wq1 = nc.dram_tensor("wq1", [D, DH], F32R, kind="ExternalInput").ap()
    wq2 = nc.dram_tensor("wq2", [D, DH], F32R, kind="ExternalInput").ap()
    wk1 = nc.dram_tensor("wk1", [D, DH], F32R, kind="ExternalInput").ap()
    wk2 = nc.dram_tensor("wk2", [D, DH], F32R, kind="ExternalInput").ap()
    wv = nc.dram_tensor("wv", [D, DH], F32R, kind="ExternalInput").ap()
    wo = nc.dram_tensor("wo", [DH, D], F32R, kind="ExternalInput").ap()
    wm = nc.dram_tensor("wm", [D, DM], FP8, kind="ExternalInput").ap()
    wn = nc.dram_tensor("wn", [D, DM], FP8, kind="ExternalInput").ap()
    wp = nc.dram_tensor("wp", [DM, D], FP8, kind="ExternalInput").ap()
    outT = nc.dram_tensor("outT", [D, R], F32, kind="ExternalOutput").ap()

    with tile.TileContext(nc) as tc:
        with tc.tile_pool(name="glob", bufs=1) as glob, \
             tc.tile_pool(name="tmp", bufs=3) as tmp:

            ident = glob.tile([P, P], F32, tag="ident")
            make_identity(nc, ident)
            ones_f = glob.tile([P, 1], F32, tag="ones_f")
            nc.vector.memset(ones_f, 1.0)
            ones = glob.tile([P, 1], F32R, tag="ones")
            nc.vector.tensor_copy(out=ones, in_=ones_f)
            eps_t = glob.tile([1, 1], F32, tag="eps")
            nc.vector.memset(eps_t, EPS)
            out1T = [glob.tile([P, R], F32, tag=f"out1T{f}", name=f"out1T{f}")
                     for f in range(FC)]

            with tc.tile_pool(name="attn", bufs=1) as attn:
                k1Tb = [attn.tile([DH, NT], F32R, tag=f"k1T{j}", name=f"k1T{j}")
                        for j in range(S // NT)]
                k2Tb = [attn.tile([DH, NT], F32R, tag=f"k2T{j}", name=f"k2T{j}")
                        for j in range(S // NT)]
                q1Tb = [attn.tile([DH, NT], F32R, tag=f"q1T{j}", name=f"q1T{j}")
                        for j in range(R // NT)]
                q2Tb = [attn.tile([DH, NT], F32R, tag=f"q2T{j}", name=f"q2T{j}")
                        for j in range(R // NT)]
                v_rm = [attn.tile([P, DH], F32R, tag=f"vrm{i}", name=f"vrm{i}")
                        for i in range(S // P)]
                attnT = attn.tile([DH, R], F32R, tag="attnT")

                with tc.tile_pool(name="xs", bufs=2) as xs, \
                     tc.tile_pool(name="wks", bufs=1) as wks, \
                     tc.tile_pool(name="sc", bufs=4) as sc, \
                     tc.tile_pool(name="psA", bufs=1, space="PSUM") as psA:

                    wblks = {}
                    for nm, w in [("wq1", wq1), ("wq2", wq2), ("wk1", wk1),
                                  ("wk2", wk2), ("wv", wv)]:
                        t = wks.tile([P, FC, DH], F32R, tag=nm, name=nm)
                        nc.gpsimd.dma_start(
                            out=t, in_=w.rearrange("(ko p) m -> p ko m", p=P))
                        wblks[nm] = t

                    # avp: attention-output accumulators, 2 cols-512 halves
                    avp = [psA.tile([P, NT], F32, tag=f"av{j}", name=f"av{j}",
                                    bufs=1)
                           for j in range(R // NT)]

                    def do_block(x_dram, cos_d, sin_d, sl, projs, tbase,
                                 xtag="xb"):
                        """One 512-col block: norm factor + projections."""
                        xr = x_dram.rearrange("(ko p) n -> p ko n", p=P)
                        xb = xs.tile([P, FC, NT], F32R, tag=xtag, name=xtag,
                                     bufs=2)
                        nc.sync.dma_start(out=xb, in_=xr[:, :, sl])
                        # norm factor r = rsqrt(mean(x^2)+eps) for this block
                        rp = psA.tile([1, NT], F32, tag="rp", bufs=1)
                        for f in range(FC):
                            sq = tmp.tile([P, NT], F32R, tag="sqr", bufs=3)
                            sf = xb[:, f].bitcast(F32)
                            if f < 4:
                                nc.scalar.activation(
                                    out=sq, in_=sf,
                                    func=mybir.ActivationFunctionType.Square,
                                    bias=0.0, scale=1.0)
                            elif f < 6:
                                nc.vector.tensor_mul(out=sq, in0=sf, in1=sf)
                            else:
                                nc.gpsimd.tensor_mul(out=sq, in0=sf, in1=sf)
                            nc.tensor.matmul(rp, ones, sq,
                                             start=(f == 0), stop=(f == FC - 1))
                        rsb = tmp.tile([1, NT], F32, tag="rsb", bufs=2)
                        nc.scalar.activation(out=rsb, in_=rp,
                                             func=mybir.ActivationFunctionType.Sqrt,
                                             bias=eps_t, scale=1.0 / D)
                        rsb2 = tmp.tile([1, NT], F32, tag="rsb2", bufs=2)
                        nc.vector.reciprocal_approx_fast(out=rsb2, in_=rsb)
                        rbb = xs.tile([P, NT], F32, tag="rbb", bufs=2)
                        nc.gpsimd.partition_broadcast(rbb, rsb2)
                        # rope tables for this block, pre-scaled by r
                        cosb = xs.tile([DH, NT], F32, tag="cosb")
                        nc.sync.dma_start(out=cosb, in_=cos_d[:, sl])
                        sinb = xs.tile([DH, NT], F32, tag="sinb")
                        nc.sync.dma_start(out=sinb, in_=sin_d[:, sl])
                        cosr = xs.tile([DH, NT], F32, tag="cosr")
                        nc.gpsimd.tensor_mul(out=cosr, in0=cosb, in1=rbb)
                        sinr = xs.tile([DH, NT], F32, tag="sinr")
                        nc.gpsimd.tensor_mul(out=sinr, in0=sinb, in1=rbb)

                        for wname, dst, kind in projs:
                            pp = psA.tile([P, NT], F32, tag="pp", bufs=2)
                            wb = wblks[wname]
                            for f in range(FC):
                                nc.tensor.matmul(pp, wb[:, f], xb[:, f],
                                                 start=(f == 0), stop=(f == FC - 1))
                            if kind == "rope":
                                t1 = tmp.tile([P, NT], F32, tag="t1")
                                nc.vector.tensor_mul(out=t1, in0=pp, in1=cosr)
                                rot = tmp.tile([P, NT], F32, tag="rot")
                                nc.scalar.activation(
                                    out=rot[0:64], in_=pp[64:128],
                                    func=mybir.ActivationFunctionType.Copy,
                                    bias=0.0, scale=1.0)
                                nc.scalar.activation(
                                    out=rot[64:128], in_=pp[0:64],
                                    func=mybir.ActivationFunctionType.Copy,
                                    bias=0.0, scale=1.0)
                                nc.vector.tensor_mul(out=rot, in0=rot, in1=sinr)
                                nc.vector.tensor_add(out=dst, in0=t1, in1=rot)
                            else:  # v: scale + transpose to row-major blocks
                                vt = tmp.tile([P, NT], F32, tag="t1")
                                nc.vector.tensor_mul(out=vt, in0=pp, in1=rbb)
                                for t in range(NT // P):
                                    tp = psA.tile([P, P], F32, tag="tp", bufs=1)
                                    nc.tensor.transpose(tp, vt[:, t * P:(t + 1) * P],
                                                        ident)
                                    nc.scalar.activation(
                                        out=v_rm[tbase + t], in_=tp,
                                        func=mybir.ActivationFunctionType.Copy,
                                        bias=0.0, scale=1.0)

                    def do_scores(kb):
                        """Score + AV for key block kb against q cols
                        [QT*(kb//4), R), in <=512 chunks."""
                        kbp = kb // 4
                        q0 = QT * kbp
                        kj, ko = kb // 4, (kb % 4) * P
                        mk = sc.tile([P, QT], BF16, tag="mk", bufs=4)
                        nc.sync.dma_start(out=mk,
                                          in_=maskP[kb * P:(kb + 1) * P, :])
                        c = q0
                        while c < R:
                            w = min(NT - c % NT, R - c)
                            hj = c // NT
                            csl = slice(c % NT, c % NT + w)
                            s1 = psA.tile([P, NT], F32, tag="s1", bufs=1)
                            s2 = psA.tile([P, NT], F32, tag="s2", bufs=1)
                            nc.tensor.matmul(s1[:, :w], k1Tb[kj][:, ko:ko + P],
                                             q1Tb[hj][:, csl],
                                             start=True, stop=True)
                            nc.tensor.matmul(s2[:, :w], k2Tb[kj][:, ko:ko + P],
                                             q2Tb[hj][:, csl],
                                             start=True, stop=True)
                            sm = tmp.tile([P, NT], F32, tag="sm", bufs=4)
                            if c == q0:
                                # diagonal 256 cols: apply mask
                                nc.vector.tensor_mul(out=sm[:, :QT],
                                                     in0=s1[:, :QT], in1=mk)
                                if w > QT:
                                    nc.scalar.activation(
                                        out=sm[:, QT:w], in_=s1[:, QT:w],
                                        func=mybir.ActivationFunctionType.Copy,
                                        bias=0.0, scale=1.0)
                            else:
                                nc.scalar.activation(
                                    out=sm[:, :w], in_=s1[:, :w],
                                    func=mybir.ActivationFunctionType.Copy,
                                    bias=0.0, scale=1.0)
                            aT = sc.tile([P, NT], F32R, tag="aT", bufs=4)
                            nc.vector.tensor_mul(out=aT[:, :w], in0=sm[:, :w],
                                                 in1=s2[:, :w])
                            nc.tensor.matmul(avp[hj][:, csl], v_rm[kb],
                                             aT[:, :w], start=(kb == 0),
                                             stop=(kb == S // P - 1),
                                             skip_group_check=True)
                            c += w

                    for jb in range(R // NT):
                        sl = slice(jb * NT, (jb + 1) * NT)
                        do_block(xqT, cos_q, sin_q, sl,
                                 [("wq1", q1Tb[jb], "rope"),
                                  ("wq2", q2Tb[jb], "rope")],
                                 tbase=0, xtag="xb")
                    for jb in range(S // NT):
                        sl = slice(jb * NT, (jb + 1) * NT)
                        do_block(xT, cos_kv, sin_kv, sl,
                                 [("wk1", k1Tb[jb], "rope"),
                                  ("wk2", k2Tb[jb], "rope"),
                                  ("wv", None, "v")], tbase=jb * (NT // P))
                        for kb in range(4 * jb, KBC[jb]):
                            do_scores(kb)

                    for hj in range(R // NT):
                        nc.vector.tensor_copy(
                            out=attnT[:, hj * NT:(hj + 1) * NT], in_=avp[hj])

                # ============ phase C: out1 = x + attn @ wo ====================
                with tc.tile_pool(name="oc", bufs=2) as oc, \
                     tc.tile_pool(name="psC", bufs=2, space="PSUM") as psC:
                    woblk = oc.tile([P, FC, P], F32R, tag="wo", bufs=1)
                    nc.gpsimd.dma_start(
                        out=woblk, in_=wo.rearrange("d (ko m) -> d ko m", m=P))
                    for f in range(FC):
                        xqr = oc.tile([P, R], F32, tag="xqr")
                        nc.gpsimd.dma_start(
                            out=xqr, in_=xqT.bitcast(F32)[f * P:(f + 1) * P, :])
                        for hj in range(R // NT):
                            sl = slice(hj * NT, (hj + 1) * NT)
                            pw = psC.tile([P, NT], F32, tag="pw")
                            nc.tensor.matmul(pw, woblk[:, f], attnT[:, sl],
                                             start=True, stop=True)
                            nc.vector.tensor_add(out=out1T[f][:, sl], in0=pw,
                                                 in1=xqr[:, sl])

            # ============ phase D: rmsnorm2 + bilinear MLP (fp8 DoubleRow) =====
            DR = mybir.MatmulPerfMode.DoubleRow
            nsl = R // NT
            with tc.tile_pool(name="mlp", bufs=1) as mlp, \
                 tc.tile_pool(name="ws", bufs=2) as ws, \
                 tc.tile_pool(name="tmpd", bufs=2) as tmpd:

                xn8h = [mlp.tile([P, FC, NT], FP8, tag=f"xn8_{j}", name=f"xn8_{j}")
                        for j in range(nsl)]
                with tc.tile_pool(name="psR", bufs=1, space="PSUM") as psR:
                    for j in range(nsl):
                        slj = slice(j * NT, (j + 1) * NT)
                        acc = psR.tile([1, NT], F32, tag="rs", bufs=2)
                        for f in range(FC):
                            sq = tmpd.tile([P, NT], F32R, tag="sq2", bufs=2)
                            nc.scalar.activation(
                                out=sq, in_=out1T[f][:, slj],
                                func=mybir.ActivationFunctionType.Square,
                                bias=0.0, scale=1.0)
                            nc.tensor.matmul(acc, ones, sq,
                                             start=(f == 0), stop=(f == FC - 1))
                        r2s = tmpd.tile([1, NT], F32, tag="r2sb", bufs=2)
                        nc.scalar.activation(out=r2s, in_=acc,
                                             func=mybir.ActivationFunctionType.Sqrt,
                                             bias=eps_t, scale=1.0 / D)
                        r2r = tmpd.tile([1, NT], F32, tag="r2r", bufs=2)
                        nc.vector.reciprocal_approx_fast(out=r2r, in_=r2s)
                        rb2 = mlp.tile([P, NT], F32, tag=f"rb2_{j}",
                                       name=f"rb2_{j}")
                        nc.gpsimd.partition_broadcast(rb2, r2r)
                        for f in range(FC):
                            nc.vector.tensor_mul(out=xn8h[j][:, f],
                                                 in0=out1T[f][:, slj],
                                                 in1=rb2)

                gts = mlp.tile([P, DM // P, R], FP8, tag="gts")
                with tc.tile_pool(name="psD", bufs=2, space="PSUM") as psD:
                    for dmc in range(DM // P):
                        wmblk = ws.tile([P, FC, P], FP8, tag="wm")
                        nc.sync.dma_start(
                            out=wmblk,
                            in_=wm[:, dmc * P:(dmc + 1) * P]
                            .rearrange("(ko p) m -> p ko m", p=P))
                        wnblk = ws.tile([P, FC, P], FP8, tag="wn")
                        nc.sync.dma_start(
                            out=wnblk,
                            in_=wn[:, dmc * P:(dmc + 1) * P]
                            .rearrange("(ko p) m -> p ko m", p=P))
                        for hj in range(nsl):
                            sl = slice(hj * NT, (hj + 1) * NT)
                            mps = psD.tile([P, NT], F32, tag="mps")
                            nps = psD.tile([P, NT], F32, tag="nps")
                            for c in range(FC // 2):
                                nc.tensor.matmul(mps, wmblk[:, 2 * c:2 * c + 2],
                                                 xn8h[hj][:, 2 * c:2 * c + 2],
                                                 start=(c == 0),
                                                 stop=(c == FC // 2 - 1),
                                                 perf_mode=DR)
                            for c in range(FC // 2):
                                nc.tensor.matmul(nps, wnblk[:, 2 * c:2 * c + 2],
                                                 xn8h[hj][:, 2 * c:2 * c + 2],
                                                 start=(c == 0),
                                                 stop=(c == FC // 2 - 1),
                                                 perf_mode=DR)
                            mcp = tmpd.tile([P, NT], BF16, tag="mcp")
                            nc.scalar.activation(
                                out=mcp, in_=mps,
                                func=mybir.ActivationFunctionType.Copy,
                                bias=0.0, scale=0.125)
                            nc.vector.tensor_mul(out=gts[:, dmc, sl],
                                                 in0=mcp, in1=nps)

                    # wp pass: accumulate all 32 dm chunks (16 DR matmuls) in
                    # PSUM per (f, hj) output tile
                    with tc.tile_pool(name="psW", bufs=1, space="PSUM") as psW:
                        for f in range(FC):
                            wpf = ws.tile([P, DM // P, P], FP8, tag="wpf")
                            nc.sync.dma_start(
                                out=wpf,
                                in_=wp[:, f * P:(f + 1) * P]
                                .rearrange("(ko p) m -> p ko m", p=P))
                            for hj in range(nsl):
                                sl = slice(hj * NT, (hj + 1) * NT)
                                wps = psW.tile([P, NT], F32, tag="wps", bufs=4)
                                for c in range(DM // P // 2):
                                    nc.tensor.matmul(
                                        wps, wpf[:, 2 * c:2 * c + 2],
                                        gts[:, 2 * c:2 * c + 2, sl],
                                        start=(c == 0),
                                        stop=(c == DM // P // 2 - 1),
                                        perf_mode=DR)
                                mlpt = tmpd.tile([P, NT], F32, tag="mlpt")
                                nc.scalar.activation(
                                    out=mlpt, in_=wps,
                                    func=mybir.ActivationFunctionType.Copy,
                                    bias=0.0, scale=1.0 / 64.0)
                                fin = tmpd.tile([P, NT], F32, tag="fin")
                                nc.vector.tensor_add(
                                    out=fin, in0=mlpt,
                                    in1=out1T[f][:, sl])
                                nc.gpsimd.dma_start(
                                    out=outT[f * P:(f + 1) * P, sl], in_=fin)

    nc.compile()
    return nc


def _get_program():
    if "nc" not in _cached:
        _cached["nc"] = _build()
    return _cached["nc"]


def kernel(x, cos, sin, causal_mask, wq1, wq2, wk1, wk2, wv, wo, wm, wn, wp):
    global LAST_EXEC_NS
    x = np.asarray(x, dtype=np.float32)
    cos = np.asarray(cos, dtype=np.float32)
    sin = np.asarray(sin, dtype=np.float32)
    causal_mask = np.asarray(causal_mask)
    B = x.shape[0]
    scale = 1.0 / np.sqrt(DH)

    coscat = np.concatenate([cos, cos], axis=1).T.copy()          # [128, S]
    sincat = np.concatenate([-sin, sin], axis=1).T.copy()         # [128, S]
    mask_val = np.where(causal_mask, 0.0, 1.0).astype(np.float32)  # [S, S]

    def to8(a):
        return np.clip(np.asarray(a, np.float32) * 8.0, -240.0, 240.0).astype(
            ml_dtypes.float8_e4m3)
    wm8, wn8, wp8 = to8(wm), to8(wn), to8(wp)

    nc = _get_program()
    in_maps = []
    rows_by_h = []
    for h in range(2):
        rows = np.concatenate([np.arange(QT * g, QT * (g + 1))
                               for g in GSET[h]])
        rows_by_h.append(rows)
    for c in range(8):
        b, h = c // 2, c % 2
        rows = rows_by_h[h]
        xb = x[b]
        mp = np.zeros((16 * P, QT), np.float32)
        for kb in range(16):
            g = GSET[h][kb // 4]
            qr = slice(QT * g, QT * (g + 1))
            kr = slice(P * kb, P * (kb + 1))
            mp[kb * P:(kb + 1) * P, :] = mask_val[qr, kr].T
        in_maps.append({
            "xT": np.ascontiguousarray(xb.T),
            "xqT": np.ascontiguousarray(xb[rows].T),
            "cos_kv": coscat,
            "sin_kv": sincat,
            "cos_q": np.ascontiguousarray(coscat[:, rows] * scale),
            "sin_q": np.ascontiguousarray(sincat[:, rows] * scale),
            "maskP": mp.astype(ml_dtypes.bfloat16),
            "wq1": np.asarray(wq1, np.float32), "wq2": np.asarray(wq2, np.float32),
            "wk1": np.asarray(wk1, np.float32), "wk2": np.asarray(wk2, np.float32),
            "wv": np.asarray(wv, np.float32), "wo": np.asarray(wo, np.float32),
            "wm": wm8, "wn": wn8, "wp": wp8,
        })

    trace = bool(os.environ.get("BASSK_TRACE"))
    if trace:
        _install_trace_hook()
    res = run_bass_kernel_spmd(nc, in_maps, core_ids=list(range(8)), trace=trace)
    LAST_EXEC_NS = res.exec_time_ns

    out = np.empty((B, S, D), dtype=np.float32)
    for c in range(8):
        b, h = c // 2, c % 2
        out[b, rows_by_h[h], :] = res.results[c]["outT"].T
    return out


def _install_trace_hook():
    import types
    import antenv
    if getattr(antenv, "axon_hooks", None) is not None:
        return
    holder = {}
    m = types.ModuleType("antenv.axon_hooks")
    m.set_axon_ntff_profile_hook = lambda h: holder.__setitem__('h', h)
    m.get_axon_ntff_profile_hook = lambda: holder.get('h')
    sys.modules["antenv.axon_hooks"] = m
    antenv.axon_hooks = m
    from trn_agent_boot.trn_boot import _ntff_profile_via_ctypes
    m.set_axon_ntff_profile_hook(_ntff_profile_via_ctypes('/opt/axon/libaxon_pjrt.so'))
